# revision 8
# baseline (speedup 1.0000x reference)
"""Trainium2 Bass kernel for CoupledStateSpaceVI.

Computes (total_elbo, P_final, x, p, u, y_pred) from full inputs, sharded
data-parallel over the time axis across 8 NeuronCores.

Key algebraic restructuring: the 127-step implicit-Euler Riccati scan is an
affine recursion P_{n+1} = M(P_n) + c with a constant linear operator
M = I + L + L^2 + L^3, where L(X) = a^T X + X a and a = DT*A_aug.  Hence

    P_127 = psi(L)(P_0) + chi(L)(DT*Q_aug)

with psi(u) = (1+u+u^2+u^3)^127 and chi(u) = (psi(u)-1)/u.  Since
||L|| ~ 4e-3, psi/chi truncate at degree J=10 far below fp32 resolution,
so the scan becomes two ~10-step operator-Horner chains (~40 matmuls).
The psi/chi coefficients are universal integers (input-independent).
"""

import numpy as np

import concourse.bass as bass
import concourse.mybir as mybir
import concourse.tile as tile
from concourse.bass_utils import run_bass_kernel_spmd
from concourse.masks import make_identity

F32 = mybir.dt.float32

N_CORES = 8
T = 131072
ROWS = T // N_CORES          # rows of the time axis per core
X_DIM = 64
AUG = 128
BATCH = 128
DT = 1.0 / 1024
BETA = 0.1
LOG2PI = float(np.log(2.0 * np.pi).astype(np.float32))

F = 4                        # 128-row blocks per T-loop iteration
NIT = ROWS // (F * 128)      # 32 iterations
J = 10                       # Horner truncation order for the Riccati chains


def _psi_coeffs(j_max):
    """Coefficients 0..j_max of (1+u+u^2+u^3)^127, exact integer arithmetic."""
    coef = [1]
    base = [1, 1, 1, 1]
    n = BATCH - 1
    while n:
        if n & 1:
            out = [0] * (j_max + 1)
            for i, ci in enumerate(coef):
                if i > j_max:
                    break
                for k, bk in enumerate(base):
                    if i + k > j_max:
                        break
                    out[i + k] += ci * bk
            coef = out
        n >>= 1
        if n:
            sq = [0] * (j_max + 1)
            for i, bi in enumerate(base):
                if i > j_max:
                    break
                for k, bk in enumerate(base):
                    if i + k > j_max:
                        break
                    sq[i + k] += bi * bk
            base = sq
    return coef


def _build_nc():
    nc = bass.Bass()

    # -------- per-core I/O --------
    mu = nc.declare_dram_parameter("mu_s", [ROWS, AUG], F32, isOutput=False)
    lv = nc.declare_dram_parameter("lv_s", [ROWS, AUG], F32, isOutput=False)
    ep = nc.declare_dram_parameter("ep_s", [ROWS, AUG], F32, isOutput=False)
    mu_b = nc.declare_dram_parameter("mu_b", [BATCH, AUG], F32, isOutput=False)
    lv_b = nc.declare_dram_parameter("lv_b", [BATCH, AUG], F32, isOutput=False)
    ep_b = nc.declare_dram_parameter("ep_b", [BATCH, AUG], F32, isOutput=False)
    y_b = nc.declare_dram_parameter("y_b", [BATCH, X_DIM], F32, isOutput=False)
    ct = nc.declare_dram_parameter("ct", [X_DIM, X_DIM], F32, isOutput=False)
    ck = nc.declare_dram_parameter("ck", [AUG, AUG], F32, isOutput=False)
    daT = nc.declare_dram_parameter("daT", [AUG, AUG], F32, isOutput=False)
    akp = nc.declare_dram_parameter("akp", [AUG, J * AUG], F32, isOutput=False)
    akc = nc.declare_dram_parameter("akc", [AUG, (J - 1) * AUG], F32, isOutput=False)
    p0m = nc.declare_dram_parameter("p0m", [AUG, AUG], F32, isOutput=False)
    qx = nc.declare_dram_parameter("qx", [AUG, AUG], F32, isOutput=False)

    z_o = nc.declare_dram_parameter("z", [ROWS, AUG], F32, isOutput=True)
    yu_o = nc.declare_dram_parameter("yu", [ROWS, AUG], F32, isOutput=True)
    pf_o = nc.declare_dram_parameter("pfin", [AUG, AUG], F32, isOutput=True)
    ec_o = nc.declare_dram_parameter("ecols", [AUG, 3], F32, isOutput=True)

    mu_r = mu.rearrange("(n f p) d -> n p f d", f=F, p=128)
    lv_r = lv.rearrange("(n f p) d -> n p f d", f=F, p=128)
    ep_r = ep.rearrange("(n f p) d -> n p f d", f=F, p=128)
    z_r = z_o.rearrange("(n f p) d -> n p f d", f=F, p=128)
    yu_r = yu_o.rearrange("(n f p) d -> n p f d", f=F, p=128)

    with tile.TileContext(nc) as tc:
        with (
            tc.tile_pool(name="consts", bufs=1) as consts,
            tc.tile_pool(name="tl", bufs=3) as tl,
            tc.tile_pool(name="ric", bufs=2) as ric,
            tc.tile_pool(name="elb", bufs=1) as elb,
            tc.tile_pool(name="ps2", bufs=2, space="PSUM") as ps2,
            tc.tile_pool(name="ps1", bufs=1, space="PSUM") as ps1,
        ):
            # ---------------- constants ----------------
            # Every tile the PE reads is staged through a DVE copy so each
            # matmul carries at most one (merged, DVE) semaphore wait —
            # walrus rejects Matmult instructions with >1 sync wait.
            def dve_stage(src_dram, shape, tag):
                raw = consts.tile(shape, F32, tag="raw_" + tag)
                nc.sync.dma_start(out=raw, in_=src_dram[:, :])
                staged = consts.tile(shape, F32, tag=tag)
                nc.vector.tensor_copy(staged, raw)
                return staged

            ident0 = consts.tile([128, 128], F32, tag="raw_id")
            make_identity(nc, ident0)
            ident = consts.tile([128, 128], F32, tag="id")
            nc.vector.tensor_copy(ident, ident0)
            ck_sb = dve_stage(ck, [AUG, AUG], "ck")
            ct_sb = dve_stage(ct, [X_DIM, X_DIM], "ct")
            daT_sb = dve_stage(daT, [AUG, AUG], "daT")
            akp_sb = dve_stage(akp, [AUG, J * AUG], "akp")
            akc_sb = dve_stage(akc, [AUG, (J - 1) * AUG], "akc")
            p0_sb = dve_stage(p0m, [AUG, AUG], "p0")
            qx_sb = dve_stage(qx, [AUG, AUG], "qx")

            # ---------------- Riccati: two operator-Horner chains ----------
            def horner_chain(x_tile, ak_sb, nsteps, tag):
                s_cur = x_tile
                for k in range(nsteps - 1, -1, -1):
                    ak = ak_sb[:, k * AUG:(k + 1) * AUG]
                    ps = ps1.tile([AUG, AUG], F32, tag="ps_" + tag)
                    nc.tensor.matmul(ps, ak, s_cur, start=True, stop=False)
                    nc.tensor.matmul(ps, s_cur, ak, start=False, stop=True)
                    s_new = ric.tile([AUG, AUG], F32, tag="s_" + tag)
                    nc.vector.tensor_add(s_new, x_tile, ps)
                    s_cur = s_new
                return s_cur

            s_psi = horner_chain(p0_sb, akp_sb, J, "p")
            s_chi = horner_chain(qx_sb, akc_sb, J - 1, "c")
            pf_sb = ric.tile([AUG, AUG], F32, tag="pf")
            nc.vector.tensor_add(pf_sb, s_psi, s_chi)
            nc.sync.dma_start(out=pf_o[:, :], in_=pf_sb)

            # ---------------- ELBO over the first BATCH rows ----------------
            mub_sb = elb.tile([BATCH, AUG], F32)
            nc.sync.dma_start(out=mub_sb, in_=mu_b[:, :])
            lvb_sb = elb.tile([BATCH, AUG], F32)
            nc.sync.dma_start(out=lvb_sb, in_=lv_b[:, :])
            epb_sb = elb.tile([BATCH, AUG], F32)
            nc.sync.dma_start(out=epb_sb, in_=ep_b[:, :])
            yb_sb = elb.tile([BATCH, X_DIM], F32)
            nc.sync.dma_start(out=yb_sb, in_=y_b[:, :])

            ecols = elb.tile([128, 3], F32)

            # z_b = mu_b + eps_b * exp(0.5 lv_b)
            sb = elb.tile([BATCH, AUG], F32)
            nc.scalar.activation(sb, lvb_sb, mybir.ActivationFunctionType.Exp,
                                 scale=0.5)
            zb = elb.tile([BATCH, AUG], F32)
            nc.vector.tensor_mul(sb, epb_sb, sb)
            nc.vector.tensor_add(zb, mub_sb, sb)

            zbT_ps = ps1.tile([AUG, BATCH], F32, tag="ps_zbT")
            nc.tensor.transpose(zbT_ps, zb, ident)
            zbT = elb.tile([AUG, BATCH], F32)
            nc.vector.tensor_copy(zbT, zbT_ps)

            # recon: err = y_b - x_b @ C^T ; sum(err^2) per partition
            xc_ps = ps1.tile([BATCH, X_DIM], F32, tag="ps_el")
            nc.tensor.matmul(xc_ps, zbT[0:X_DIM, :], ct_sb)
            err = elb.tile([BATCH, X_DIM], F32)
            nc.vector.tensor_sub(err, yb_sb, xc_ps)
            err_sq = elb.tile([BATCH, X_DIM], F32)
            nc.scalar.activation(err_sq, err, mybir.ActivationFunctionType.Square,
                                 accum_out=ecols[:, 0:1])

            # dynamics in transposed layout: derr^T = zbT[:,1:] - zbT[:,:-1]
            #   - DT*A_aug @ zbT[:,:-1]
            dyn_ps = ps1.tile([AUG, BATCH - 1], F32, tag="ps_el")
            nc.tensor.matmul(dyn_ps, daT_sb, zbT[:, 0:BATCH - 1])
            d1 = elb.tile([AUG, BATCH - 1], F32)
            nc.vector.tensor_sub(d1, zbT[:, 1:BATCH], zbT[:, 0:BATCH - 1])
            nc.vector.tensor_sub(d1, d1, dyn_ps)
            d1_sq = elb.tile([AUG, BATCH - 1], F32)
            nc.scalar.activation(d1_sq, d1, mybir.ActivationFunctionType.Square,
                                 accum_out=ecols[:, 1:2])

            # KL: sum(1 + lv - mu^2 - exp(lv)) per partition
            eb = elb.tile([BATCH, AUG], F32)
            nc.scalar.activation(eb, lvb_sb, mybir.ActivationFunctionType.Exp)
            k1 = elb.tile([BATCH, AUG], F32)
            nc.vector.tensor_scalar_add(k1, lvb_sb, 1.0)
            nc.vector.tensor_sub(k1, k1, eb)
            mu2 = elb.tile([BATCH, AUG], F32)
            nc.vector.tensor_mul(mu2, mub_sb, mub_sb)
            nc.vector.tensor_sub(k1, k1, mu2)
            nc.vector.reduce_sum(out=ecols[:, 2:3], in_=k1,
                                 axis=mybir.AxisListType.X)
            nc.sync.dma_start(out=ec_o[:, :], in_=ecols)

            # ---------------- main T loop ----------------
            for i in range(NIT):
                mu_t = tl.tile([128, F, AUG], F32)
                nc.sync.dma_start(out=mu_t, in_=mu_r[i])
                lv_t = tl.tile([128, F, AUG], F32)
                nc.sync.dma_start(out=lv_t, in_=lv_r[i])
                ep_t = tl.tile([128, F, AUG], F32)
                nc.sync.dma_start(out=ep_t, in_=ep_r[i])

                st = tl.tile([128, F, AUG], F32)
                nc.scalar.activation(st, lv_t, mybir.ActivationFunctionType.Exp,
                                     scale=0.5)
                zt = tl.tile([128, F, AUG], F32)
                nc.vector.tensor_mul(st, ep_t, st)
                nc.vector.tensor_add(zt, mu_t, st)
                nc.sync.dma_start(out=z_r[i], in_=zt)

                zT_ps = ps2.tile([128, F, AUG], F32)
                for f in range(F):
                    nc.tensor.transpose(zT_ps[:, f], zt[:, f], ident)
                zTs = tl.tile([128, F, AUG], F32)
                nc.vector.tensor_copy(zTs, zT_ps)

                yu_ps = ps2.tile([128, F, AUG], F32)
                for f in range(F):
                    nc.tensor.matmul(yu_ps[:, f], zTs[:, f], ck_sb)
                yu_sb = tl.tile([128, F, AUG], F32)
                nc.vector.tensor_copy(yu_sb, yu_ps)
                nc.sync.dma_start(out=yu_r[i], in_=yu_sb)

    _split_waits(nc)
    return nc


def _split_waits(nc):
    """This walrus build rejects engine instructions carrying more than one
    sync wait (single wait slot per ISA struct).  Relocate all-but-one waits
    of any multi-wait instruction onto same-engine no-op carriers inserted
    immediately before it: engines execute their queues in order, so a wait
    executed on a preceding no-op gates the instruction identically."""
    n = 0
    for b in nc.m.functions[0].blocks:
        insts = b.instructions          # live list view
        edits = []
        for pos, i in enumerate(insts):
            si = i.sync_info
            if si is None or len(si.on_wait) <= 1:
                continue
            if isinstance(i, mybir.InstEventSemaphore):
                continue
            edits.append((pos, i, list(si.on_wait), list(si.on_update)))
        for pos, i, waits, updates in reversed(edits):
            carriers = []
            for w in waits[:-1]:
                nop = mybir.InstEventSemaphore(name=f"W-split-{n}", ins=[],
                                               outs=[])
                n += 1
                nop.engine = i.engine
                nop.sync_info = mybir.SyncInfo(on_wait=[w], on_update=[])
                carriers.append(nop)
            i.sync_info = mybir.SyncInfo(on_wait=[waits[-1]], on_update=updates)
            for c in reversed(carriers):
                insts.insert(pos, c)


_NC = None


def _get_nc():
    global _NC
    if _NC is None:
        _NC = _build_nc()
    return _NC


def _prep_inputs(y, C, A, B, Q, R, q_mu, q_logvar, P0, eps_batch, eps_full):
    f32 = np.float32
    C = np.asarray(C, f32)
    A = np.asarray(A, f32)
    B = np.asarray(B, f32)
    Q = np.asarray(Q, f32)
    R = np.asarray(R, f32)
    P0 = np.asarray(P0, f32)

    R_inv = np.linalg.inv(R)
    BRB = B @ R_inv @ B.T
    A_aug = np.block([[A, -BRB], [-Q, -A.T]]).astype(f32)
    K = R_inv @ B.T                                    # [U, X]
    ck = np.zeros((AUG, AUG), f32)
    ck[:X_DIM, :X_DIM] = C.T                           # y_pred = x @ C^T
    ck[X_DIM:, X_DIM:] = -K.T                          # u = -(p @ K^T)

    a64 = (DT * A_aug).astype(np.float64)
    psi = _psi_coeffs(J)                               # ints, len J+1
    chi = psi[1:]                                      # chi_k = psi_{k+1}
    gam_p = [psi[k + 1] / psi[k] for k in range(J)]
    gam_c = [chi[k + 1] / chi[k] for k in range(J - 1)]
    akp = np.concatenate([(g * a64) for g in gam_p], axis=1).astype(f32)
    akc = np.concatenate([(g * a64) for g in gam_c], axis=1).astype(f32)

    Q_aug = np.zeros((AUG, AUG), np.float64)
    Q_aug[:X_DIM, :X_DIM] = Q
    qx = (float(chi[0]) * DT * Q_aug).astype(f32)      # chi_0 * DT * Q_aug

    daT = (DT * A_aug).T.astype(f32).copy()            # lhsT for dyn matmul

    shared = dict(
        mu_b=np.ascontiguousarray(q_mu[:BATCH], f32),
        lv_b=np.ascontiguousarray(q_logvar[:BATCH], f32),
        ep_b=np.ascontiguousarray(eps_batch, f32),
        y_b=np.ascontiguousarray(y[:BATCH], f32),
        ct=np.ascontiguousarray(C.T),
        ck=ck,
        daT=daT,
        akp=np.ascontiguousarray(akp),
        akc=np.ascontiguousarray(akc),
        p0m=np.ascontiguousarray(P0),
        qx=qx,
    )
    in_maps = []
    for c in range(N_CORES):
        sl = slice(c * ROWS, (c + 1) * ROWS)
        m = dict(shared)
        m["mu_s"] = np.ascontiguousarray(q_mu[sl], f32)
        m["lv_s"] = np.ascontiguousarray(q_logvar[sl], f32)
        m["ep_s"] = np.ascontiguousarray(eps_full[sl], f32)
        in_maps.append(m)
    return in_maps


def _assemble(results):
    z = np.concatenate([r["z"] for r in results], axis=0)
    yu = np.concatenate([r["yu"] for r in results], axis=0)
    x = np.ascontiguousarray(z[:, :X_DIM])
    p = np.ascontiguousarray(z[:, X_DIM:])
    y_pred = np.ascontiguousarray(yu[:, :X_DIM])
    u = np.ascontiguousarray(yu[:, X_DIM:])
    p_final = results[0]["pfin"]
    ec = results[0]["ecols"].astype(np.float64)
    recon_sum = ec[:, 0].sum()
    dyn_sum = ec[:, 1].sum()
    kl_sum = ec[:, 2].sum()
    recon_loss = -0.5 * (recon_sum + BATCH * X_DIM * LOG2PI)
    dyn_loss = -0.5 * (dyn_sum + (BATCH - 1) * AUG * LOG2PI)
    kl = -0.5 * kl_sum
    total = np.float32(recon_loss + dyn_loss - BETA * kl)
    return total, p_final, x, p, u, y_pred


def _run(inputs, trace=False, **kw):
    nc = _get_nc()
    in_maps = _prep_inputs(**inputs)
    bkr = run_bass_kernel_spmd(nc, in_maps, core_ids=list(range(N_CORES)),
                               trace=trace, **kw)
    return _assemble(bkr.results), bkr


def kernel(**inputs):
    out, _ = _run(inputs)
    return out
